# revision 6
# baseline (speedup 1.0000x reference)
"""Trainium2 Bass kernel for ClassicalGCN message passing.

Reference computation:
    h   = tanh(x @ W1 + b1)                       # [N, HID]
    agg = segment_sum(edge_val * h[edge_col], edge_row, N)
    out = agg @ W2 + b2                           # [N, 1]

Key algebraic rewrite: W2 commutes through the linear aggregation:

    s      = tanh(x @ W1 + b1) @ W2               # [N] per-node scalar
    out[i] = b2 + sum_{e: row[e]==i} val[e] * s[col[e]]

Sharding: nodes (output rows) are split across the 8 cores; edges are
partitioned by destination row. x and the small weights are replicated; each
core computes the full s vector locally (no collectives) and then
aggregates only its own edges.

Per-core device program:
  Phase A: s = tanh(x@W1+b1)@W2 for all nodes via PE matmuls (W1 as the
           stationary operand streaming x^T), ACT tanh (bias fused), PE
           W2-contraction; s spilled to a DRAM scratch table.
  Phase B: ELL layout (w=40 slots/row). Per edge slot the kernel
           dma_gathers the 256-byte s-block containing the needed column
           (block = col>>6; the s table is viewed as [784, 64] so block
           indices fit the gather's int16 index format), multiplies by a
           host-built f32 mask (val at offset col%64, zero elsewhere —
           also zero for padding slots), and reduces (slot, 64) per row.

Rows with degree > 40 overflow to an exact host-side fixup (~0.7% of
edges). b2 and the final stitch-up happen on the host.
"""

import os

import numpy as np

import concourse.bass as bass
import concourse.mybir as mybir
import concourse.tile as tile
from concourse import bacc
from concourse.bass_utils import run_bass_kernel_spmd
from concourse.tile_rust import add_dep_helper

# Problem sizes (hardcoded per spec nn_ClassicalGCN_77077483094916)
N = 50000
E = 1600000
IN_DIM = 128
HID = 64
NCORES = 8

RPC = N // NCORES            # rows per core = 6250
RPAD = 6272                  # rows padded to 128*49
ROWS_F = RPAD // 128         # 49 rows per partition
NPAD = 50176                 # nodes padded to 98*512 = 784*64
NBLK = NPAD // 64            # 784 s-blocks of 64 (256B each)
ACHUNKS = NPAD // 1024       # 49 phase-A iterations
W_ELL = 40                   # edge slots per row on device
# phase-B chunks: rows-per-partition processed per gather
CHUNK_ROWS = [2] * 24 + [1]  # sums to 49

F32 = mybir.dt.float32
I16 = mybir.dt.int16

_LAST_RESULTS = {"exec_time_ns": None}


def _install_ntff_hook():
    """Register the axon NTFF profile hook if the image's antenv lacks it.

    bass_utils reads `antenv.axon_hooks.get_axon_ntff_profile_hook` lazily
    when trace=True under axon; this image's antenv has no axon_hooks
    module, so synthesize one around direct ctypes calls into
    libaxon_pjrt.so (same ABI trn_boot._ntff_profile_via_ctypes uses).
    """
    import sys

    if "antenv.axon_hooks" in sys.modules:
        return
    import contextlib
    import ctypes
    import types

    so_path = "/opt/axon/libaxon_pjrt.so"
    if not os.path.exists(so_path):
        return
    try:
        lib = ctypes.CDLL(so_path)
    except OSError:
        return
    if not hasattr(lib, "axon_start_nrt_profile"):
        return
    lib.axon_start_nrt_profile.argtypes = [
        ctypes.POINTER(ctypes.c_int64),
        ctypes.c_size_t,
    ]
    lib.axon_start_nrt_profile.restype = ctypes.c_int64
    lib.axon_stop_nrt_profile.argtypes = [ctypes.c_char_p]
    lib.axon_stop_nrt_profile.restype = ctypes.c_int64

    @contextlib.contextmanager
    def _hook(output_dir, device_ids):
        import jax

        jax.devices()
        if device_ids:
            ids = (ctypes.c_int64 * len(device_ids))(*device_ids)
            rc = lib.axon_start_nrt_profile(ids, len(device_ids))
        else:
            rc = lib.axon_start_nrt_profile(None, 0)
        if rc != 0:
            raise RuntimeError(f"axon_start_nrt_profile rc={rc}")
        try:
            yield
        finally:
            n = lib.axon_stop_nrt_profile(str(output_dir).encode())
            if n < 0:
                raise RuntimeError(f"axon_stop_nrt_profile rc={n}")
            print(f"profile: {n} file(s) written to {output_dir}")

    mod = types.ModuleType("antenv.axon_hooks")
    mod.get_axon_ntff_profile_hook = lambda: _hook
    mod.set_axon_ntff_profile_hook = lambda h: None
    sys.modules["antenv.axon_hooks"] = mod


def _build_program():
    FE = ROWS_F * W_ELL                  # 1960 slots per partition
    NIDX = 128 * W_ELL // 16             # idx columns per row-chunk unit
    nc = bacc.Bacc("TRN2", target_bir_lowering=False, debug=False)

    xT = nc.dram_tensor("xT", [128, NPAD], F32, kind="ExternalInput")
    W1 = nc.dram_tensor("W1", [128, HID], F32, kind="ExternalInput")
    b1c = nc.dram_tensor("b1c", [128, 1], F32, kind="ExternalInput")
    W2d = nc.dram_tensor("W2d", [128, 2], F32, kind="ExternalInput")
    blk = nc.dram_tensor("blk", [128, ROWS_F * NIDX], I16, kind="ExternalInput")
    vmask = nc.dram_tensor("vmask", [128, FE * 64], F32, kind="ExternalInput")
    outd = nc.dram_tensor("out", [128, ROWS_F], F32, kind="ExternalOutput")

    with tile.TileContext(nc) as tc:
        with (
            tc.tile_pool(name="const", bufs=1) as cpool,
            tc.tile_pool(name="dram", bufs=1, space="DRAM") as dpool,
        ):
            W1_sb = cpool.tile([128, HID], F32)
            nc.sync.dma_start(W1_sb[:], W1[:, :])
            b1_sb = cpool.tile([128, 1], F32)
            nc.sync.dma_start(b1_sb[:], b1c[:, :])
            W2_sb = cpool.tile([128, 2], F32)
            nc.sync.dma_start(W2_sb[:], W2d[:, :])

            s_dram = dpool.tile([NPAD, 1], F32)

            # ---- Phase A: s = tanh(x@W1+b1) @ W2 for all nodes ----
            with (
                tc.tile_pool(name="xload", bufs=3) as xpool,
                tc.tile_pool(name="thp", bufs=2) as thpool,
                tc.tile_pool(name="ssp", bufs=2) as sspool,
                tc.tile_pool(name="pz", bufs=2, space="PSUM") as pz,
                tc.tile_pool(name="psd", bufs=2, space="PSUM") as psd,
            ):
                for i in range(ACHUNKS):
                    xt = xpool.tile([128, 1024], F32)
                    nc.sync.dma_start(xt[:], xT[:, 1024 * i : 1024 * (i + 1)])
                    z = pz.tile([128, 512], F32)
                    nc.tensor.matmul(z[0:64, :], lhsT=W1_sb[:],
                                     rhs=xt[:, 0:512], start=True, stop=True)
                    nc.tensor.matmul(z[64:128, :], lhsT=W1_sb[:],
                                     rhs=xt[:, 512:1024], start=True, stop=True)
                    th = thpool.tile([128, 512], F32)
                    nc.scalar.activation(th[:], z[:],
                                         mybir.ActivationFunctionType.Tanh,
                                         bias=b1_sb[:, 0:1])
                    sp = psd.tile([2, 512], F32)
                    nc.tensor.matmul(sp[:], lhsT=W2_sb[:], rhs=th[:],
                                     start=True, stop=True)
                    ss = sspool.tile([2, 512], F32)
                    nc.vector.tensor_copy(ss[:], sp[:])
                    nc.sync.dma_start(
                        s_dram[1024 * i : 1024 * (i + 1), 0].rearrange(
                            "(j t) -> j t", j=2),
                        ss[:],
                    )

            s_tbl = s_dram[:, 0].rearrange("(b d) -> b d", d=64)

            # ---- Phase B: block-gather + mask multiply + reduce ----
            with (
                tc.tile_pool(name="gat", bufs=2) as gpool,
                tc.tile_pool(name="vml", bufs=2) as vpool,
                tc.tile_pool(name="ell", bufs=1) as epool,
            ):
                blk_sb = epool.tile([128, ROWS_F * NIDX], I16)
                nc.sync.dma_start(blk_sb[:], blk[:, :])
                out_sb = epool.tile([128, ROWS_F], F32)

                n0 = 0
                last_reduce = [None, None]        # per rotating g-slot
                for ci, nch in enumerate(CHUNK_ROWS):
                    ni = 128 * nch * W_ELL            # idxs this chunk
                    fch = nch * W_ELL * 64            # f32s per partition
                    g = gpool.tile([128, fch], F32, tag="g")
                    ginst = nc.gpsimd.dma_gather(
                        out_ap=g[:].rearrange("p (c d) -> p c d", d=64),
                        in_ap=s_tbl,
                        idxs_ap=blk_sb[:, n0 * NIDX : (n0 + nch) * NIDX],
                        num_idxs=ni,
                        num_idxs_reg=ni,
                        elem_size=64,
                        single_packet=False,
                    )
                    # Tile's auto-sync misses waits around DMAGatherAnt;
                    # enforce the WAR against the previous user of this slot
                    if last_reduce[ci % 2] is not None:
                        add_dep_helper(ginst.ins, last_reduce[ci % 2].ins,
                                       reason="slot reuse WAR")
                    vm = vpool.tile([128, fch], F32, tag="vm")
                    nc.sync.dma_start(
                        vm[:],
                        vmask[:, n0 * W_ELL * 64 : (n0 + nch) * W_ELL * 64],
                    )
                    minst = nc.vector.tensor_tensor(
                        out=g[:], in0=g[:], in1=vm[:], op=mybir.AluOpType.mult
                    )
                    # and the RAW gather -> first consumer
                    add_dep_helper(minst.ins, ginst.ins,
                                   reason="wait gather data")
                    rinst = nc.vector.tensor_reduce(
                        out=out_sb[:, n0 : n0 + nch],
                        in_=g[:].rearrange("p (n k d) -> p n k d",
                                           k=W_ELL, d=64),
                        axis=mybir.AxisListType.XY,
                        op=mybir.AluOpType.add,
                    )
                    last_reduce[ci % 2] = rinst
                    n0 += nch

                nc.sync.dma_start(outd[:, :], out_sb[:])
    nc.compile()
    return nc


_PROGRAM_CACHE = {}


def _get_program():
    if "p" not in _PROGRAM_CACHE:
        _PROGRAM_CACHE["p"] = _build_program()
    return _PROGRAM_CACHE["p"]


def _wrap16(idx_flat):
    """dma_gather index layout: idx i -> [i % 16, i // 16], replicated to
    128 partitions."""
    ni = idx_flat.shape[0]
    a = np.zeros((16, ni // 16), np.int16)
    a[np.arange(ni) % 16, np.arange(ni) // 16] = idx_flat
    return np.tile(a, (8, 1))


def _preprocess(x, edge_row, edge_col, edge_val, W1, b1, W2):
    xT = np.zeros((128, NPAD), np.float32)
    xT[:, :N] = x.T

    order = np.argsort(edge_row, kind="stable")
    ers = edge_row[order]
    ecs = edge_col[order].astype(np.int64)
    evs = edge_val[order]

    deg = np.bincount(ers, minlength=N)
    starts = np.zeros(N + 1, np.int64)
    np.cumsum(deg, out=starts[1:])
    pos = np.arange(E, dtype=np.int64) - starts[ers]

    main = pos < W_ELL
    # ---- device part: ELL [N, W_ELL] of (block, offset, val) ----
    ell_blk = np.zeros((N, W_ELL), np.int16)
    ell_off = np.zeros((N, W_ELL), np.int8)
    ell_val = np.zeros((N, W_ELL), np.float32)
    ell_blk[ers[main], pos[main]] = (ecs[main] >> 6).astype(np.int16)
    ell_off[ers[main], pos[main]] = (ecs[main] & 63).astype(np.int8)
    ell_val[ers[main], pos[main]] = evs[main]

    blk_cores = []
    vm_cores = []
    for k in range(NCORES):
        bk = np.zeros((RPAD, W_ELL), np.int16)
        ok = np.zeros((RPAD, W_ELL), np.int64)
        vk = np.zeros((RPAD, W_ELL), np.float32)
        bk[:RPC] = ell_blk[k * RPC : (k + 1) * RPC]
        ok[:RPC] = ell_off[k * RPC : (k + 1) * RPC]
        vk[:RPC] = ell_val[k * RPC : (k + 1) * RPC]
        # device row r = 49*p + n ; gather slot i = p + 128*(n*W + w)
        bk = bk.reshape(128, ROWS_F, W_ELL)     # [p, n, w]
        ok = ok.reshape(128, ROWS_F, W_ELL)
        vk = vk.reshape(128, ROWS_F, W_ELL)
        # idx list in slot order i = p + 128*(n*W + w): transpose to
        # [n, w, p] then flatten
        idx_flat = np.ascontiguousarray(
            bk.transpose(1, 2, 0)).reshape(-1)     # [n*w*128]
        blk_cores.append(_wrap16(idx_flat))
        # vmask[p, ((n*W + w)*64 + d)] = val if d == off else 0
        vm = np.zeros((128, ROWS_F * W_ELL, 64), np.float32)
        pp, nn, ww = np.nonzero(vk)
        vm[pp, nn * W_ELL + ww, ok[pp, nn, ww]] = vk[pp, nn, ww]
        vm_cores.append(vm.reshape(128, ROWS_F * W_ELL * 64))

    # ---- host part: overflow edges (pos >= W_ELL), exact f32 math ----
    ov = ~main
    host_add = np.zeros(N, np.float32)
    if ov.any():
        cols = ecs[ov]
        h_ov = np.tanh(x[cols] @ W1 + b1)
        s_ov = (h_ov @ W2)[:, 0]
        np.add.at(host_add, ers[ov], evs[ov] * s_ov)

    W1h = np.ascontiguousarray(W1.astype(np.float32))
    b1c = np.tile(b1.astype(np.float32), 2).reshape(128, 1)
    W2d = np.zeros((128, 2), np.float32)
    W2d[0:64, 0] = W2[:, 0]
    W2d[64:128, 1] = W2[:, 0]
    return xT, blk_cores, vm_cores, W1h, b1c, W2d, host_add


def kernel(x, edge_row, edge_col, edge_val, W1, b1, W2, b2):
    x = np.asarray(x, np.float32)
    edge_row = np.asarray(edge_row, np.int32)
    edge_col = np.asarray(edge_col, np.int32)
    edge_val = np.asarray(edge_val, np.float32)
    W1 = np.asarray(W1, np.float32)
    b1 = np.asarray(b1, np.float32)
    W2 = np.asarray(W2, np.float32)
    b2 = np.asarray(b2, np.float32)

    xT, blk_cores, vm_cores, W1h, b1c, W2d, host_add = _preprocess(
        x, edge_row, edge_col, edge_val, W1, b1, W2
    )
    nc = _get_program()

    in_maps = [
        {
            "xT": xT,
            "W1": W1h,
            "b1c": b1c,
            "W2d": W2d,
            "blk": blk_cores[k],
            "vmask": vm_cores[k],
        }
        for k in range(NCORES)
    ]
    trace = bool(int(os.environ.get("GCN_TRACE", "0")))
    if trace:
        _install_ntff_hook()
    res = run_bass_kernel_spmd(
        nc,
        in_maps,
        core_ids=list(range(NCORES)),
        trace=trace,
    )
    _LAST_RESULTS["trace"] = res.instructions_and_trace
    _LAST_RESULTS["scope_times"] = res.per_core_scope_times
    _LAST_RESULTS["exec_time_ns"] = res.exec_time_ns

    out = np.empty((N, 1), np.float32)
    for k in range(NCORES):
        o = res.results[k]["out"]            # [128, 49] partition-major rows
        out[k * RPC : (k + 1) * RPC, 0] = o.reshape(RPAD)[:RPC]
    out[:, 0] += host_add + float(b2.reshape(-1)[0])
    return out



# revision 11
# speedup vs baseline: 3.9137x; 3.9137x over previous
"""Trainium2 Bass kernel for ClassicalGCN message passing.

Reference computation:
    h   = tanh(x @ W1 + b1)                       # [N, HID]
    agg = segment_sum(edge_val * h[edge_col], edge_row, N)
    out = agg @ W2 + b2                           # [N, 1]

Key algebraic rewrite: W2 commutes through the linear aggregation:

    s      = tanh(x @ W1 + b1) @ W2               # [N] per-node scalar
    out[i] = b2 + sum_{e: row[e]==i} val[e] * s[col[e]]

Sharding: nodes (output rows) are split across the 8 NeuronCores; edges are
partitioned by destination row. x and the small weights are replicated; each
core computes the full s vector locally (no collectives), keeps it in SBUF
as a bf16 pair-table replicated across all 128 partitions, and gathers
per-edge values with the GPSIMD ap_gather ucode (SBUF-local gather; no
per-edge HBM traffic and no SWDGE descriptor generation).

Per-core device program:
  Phase A: s = tanh(x@W1+b1)@W2 for all nodes via PE matmuls, ACT tanh
           (bias fused), PE W2-contraction; each 1024-node chunk is
           converted to bf16 and broadcast from a DRAM staging buffer into
           all 128 partitions of the SBUF table [128, 25088 pairs].
  Phase B: rows are sorted by degree (host-side permutation, undone on the
           host afterwards) and processed in 49 chunks of 128 rows with a
           per-chunk ELL width W[k] = max degree in the chunk. Per chunk:
           ap_gather fetches the bf16 pair containing s[col] for each edge
           slot (idx = col>>1, 8 GPSIMD cores each serve 16 rows), a small
           SBUF->SBUF DMA per 16-partition group re-stripes the (16x
           redundant) gathered stream to one row per partition, and the DVE
           multiplies by a host-built 2-lane mask (edge val at lane col&1)
           and reduces to out[128 rows, chunk].
"""

import os

import numpy as np
import ml_dtypes

import concourse.bass as bass
import concourse.mybir as mybir
import concourse.tile as tile
from concourse import bacc
from concourse.bass_utils import run_bass_kernel_spmd
from concourse.tile_rust import add_dep_helper

# Problem sizes (hardcoded per spec nn_ClassicalGCN_77077483094916)
N = 50000
E = 1600000
IN_DIM = 128
HID = 64
NCORES = 8

RPC = N // NCORES            # rows per core = 6250
RPAD = 6272                  # rows padded to 128*49
NCHUNK = RPAD // 128         # 49 row chunks of 128
NPAD = 50176                 # nodes padded to 49*1024
NPAIR = NPAD // 2            # bf16 pairs in the s table
ACHUNKS = NPAD // 1024       # 49 phase-A iterations

F32 = mybir.dt.float32
BF16 = mybir.dt.bfloat16
I16 = mybir.dt.int16

_LAST_RESULTS = {"exec_time_ns": None}


def _install_ntff_hook():
    """Register the axon NTFF profile hook if the image's antenv lacks it.

    bass_utils reads `antenv.axon_hooks.get_axon_ntff_profile_hook` lazily
    when trace=True under axon; this image's antenv has no axon_hooks
    module, so synthesize one around direct ctypes calls into
    libaxon_pjrt.so (same ABI trn_boot._ntff_profile_via_ctypes uses).
    """
    import sys

    if "antenv.axon_hooks" in sys.modules:
        return
    import contextlib
    import ctypes
    import types

    so_path = "/opt/axon/libaxon_pjrt.so"
    if not os.path.exists(so_path):
        return
    try:
        lib = ctypes.CDLL(so_path)
    except OSError:
        return
    if not hasattr(lib, "axon_start_nrt_profile"):
        return
    lib.axon_start_nrt_profile.argtypes = [
        ctypes.POINTER(ctypes.c_int64),
        ctypes.c_size_t,
    ]
    lib.axon_start_nrt_profile.restype = ctypes.c_int64
    lib.axon_stop_nrt_profile.argtypes = [ctypes.c_char_p]
    lib.axon_stop_nrt_profile.restype = ctypes.c_int64

    @contextlib.contextmanager
    def _hook(output_dir, device_ids):
        import jax

        jax.devices()
        if device_ids:
            ids = (ctypes.c_int64 * len(device_ids))(*device_ids)
            rc = lib.axon_start_nrt_profile(ids, len(device_ids))
        else:
            rc = lib.axon_start_nrt_profile(None, 0)
        if rc != 0:
            raise RuntimeError(f"axon_start_nrt_profile rc={rc}")
        try:
            yield
        finally:
            n = lib.axon_stop_nrt_profile(str(output_dir).encode())
            if n < 0:
                raise RuntimeError(f"axon_stop_nrt_profile rc={n}")
            print(f"profile: {n} file(s) written to {output_dir}")

    mod = types.ModuleType("antenv.axon_hooks")
    mod.get_axon_ntff_profile_hook = lambda: _hook
    mod.set_axon_ntff_profile_hook = lambda h: None
    sys.modules["antenv.axon_hooks"] = mod


def _build_program(W):
    """W: tuple of NCHUNK per-chunk ELL widths (max degree in each chunk)."""
    sumW = sum(W)
    nc = bacc.Bacc("TRN2", target_bir_lowering=False, debug=False)

    xT = nc.dram_tensor("xT", [128, NPAD], BF16, kind="ExternalInput")
    W1 = nc.dram_tensor("W1", [128, HID], BF16, kind="ExternalInput")
    b1c = nc.dram_tensor("b1c", [128, 1], F32, kind="ExternalInput")
    # W2r[:, 0:128]: col m = W2 on partitions 0:64, zeros elsewhere;
    # W2r[:, 128:256]: col m = W2 on partitions 64:128. Contracting either
    # against th replicates the per-node scalar across all 128 partitions.
    W2r = nc.dram_tensor("W2r", [128, 256], BF16, kind="ExternalInput")
    idx = nc.dram_tensor("idx", [128, sumW], I16, kind="ExternalInput")
    msk = nc.dram_tensor("msk", [128, 2 * sumW], BF16, kind="ExternalInput")
    outd = nc.dram_tensor("out", [128, NCHUNK], F32, kind="ExternalOutput")

    with tile.TileContext(nc) as tc:
        with (
            tc.tile_pool(name="const", bufs=1) as cpool,
        ):
            W1_sb = cpool.tile([128, HID], BF16)
            nc.sync.dma_start(W1_sb[:], W1[:, :])
            b1_sb = cpool.tile([128, 1], F32)
            nc.sync.dma_start(b1_sb[:], b1c[:, :])
            W2_sb = cpool.tile([128, 256], BF16)
            nc.sync.dma_start(W2_sb[:], W2r[:, :])

            table_sb = cpool.tile([128, NPAD], BF16)

            # ---- Phase A: table[:, n] = tanh(x@W1+b1)@W2, all partitions --
            bcast_insts = []
            with (
                tc.tile_pool(name="xload", bufs=3) as xpool,
                tc.tile_pool(name="thp", bufs=2) as thpool,
                tc.tile_pool(name="pz", bufs=2, space="PSUM") as pz,
                tc.tile_pool(name="psd", bufs=2, space="PSUM") as psd,
            ):
                for i in range(ACHUNKS):
                    xt = xpool.tile([128, 1024], BF16)
                    nc.sync.dma_start(xt[:], xT[:, 1024 * i : 1024 * (i + 1)])
                    z = pz.tile([128, 512], F32)
                    nc.tensor.matmul(z[0:64, :], lhsT=W1_sb[:],
                                     rhs=xt[:, 0:512], start=True, stop=True)
                    nc.tensor.matmul(z[64:128, :], lhsT=W1_sb[:],
                                     rhs=xt[:, 512:1024], start=True, stop=True)
                    th = thpool.tile([128, 512], BF16)
                    nc.scalar.activation(th[:], z[:],
                                         mybir.ActivationFunctionType.Tanh,
                                         bias=b1_sb[:, 0:1])
                    sA = psd.tile([128, 512], F32, tag="sA")
                    nc.tensor.matmul(sA[:], lhsT=W2_sb[:, 0:128], rhs=th[:],
                                     start=True, stop=True)
                    sB = psd.tile([128, 512], F32, tag="sB")
                    nc.tensor.matmul(sB[:], lhsT=W2_sb[:, 128:256], rhs=th[:],
                                     start=True, stop=True)
                    cA = nc.scalar.activation(
                        table_sb[:, 1024 * i : 1024 * i + 512], sA[:],
                        mybir.ActivationFunctionType.Copy)
                    cB = nc.vector.tensor_copy(
                        table_sb[:, 1024 * i + 512 : 1024 * (i + 1)], sB[:])
                    bcast_insts.append(cA)
                    bcast_insts.append(cB)

            tblv = table_sb[:].rearrange("p (n d) -> p n d", d=2)

            # ---- Phase B: per-chunk ap_gather + restripe + mask-reduce ----
            with (
                tc.tile_pool(name="gat", bufs=2) as gpool,
                tc.tile_pool(name="rts", bufs=2) as rpool,
                tc.tile_pool(name="mks", bufs=2) as mpool,
                tc.tile_pool(name="ell", bufs=1) as epool,
            ):
                idx_sb = epool.tile([128, sumW], I16)
                idx_ld = nc.sync.dma_start(idx_sb[:], idx[:, :])
                out_sb = epool.tile([128, NCHUNK], F32)

                offW = 0
                last_rds = [None, None]      # per g slot: restripe DMAs
                last_red = [None, None]      # per rt slot: reduce inst
                for k in range(NCHUNK):
                    Wk = W[k]
                    ni = 16 * Wk
                    g = gpool.tile([128, ni * 2], BF16, tag="g")
                    gi = nc.gpsimd.ap_gather(
                        out_ap=g[:].rearrange("p (n d) -> p n d", d=2),
                        in_ap=tblv,
                        idxs_ap=idx_sb[:, offW : offW + Wk],
                        channels=128,
                        num_elems=NPAIR,
                        d=2,
                        num_idxs=ni,
                    )
                    if k == 0:
                        for bi in bcast_insts:
                            add_dep_helper(gi.ins, bi.ins, reason="table RAW")
                        add_dep_helper(gi.ins, idx_ld.ins, reason="idx RAW")
                    if last_rds[k % 2] is not None:
                        for rd in last_rds[k % 2]:
                            add_dep_helper(gi.ins, rd.ins, reason="g WAR")

                    rt = rpool.tile([128, Wk * 2], BF16, tag="rt")
                    rds = []
                    gv = g[:].rearrange("p (s wl) -> p s wl", s=16)
                    for c in range(8):
                        rd = nc.sync.dma_start(
                            rt[16 * c : 16 * (c + 1), :],
                            gv[16 * c : 16 * c + 1, :, :],
                        )
                        add_dep_helper(rd.ins, gi.ins, reason="gather RAW")
                        if last_red[k % 2] is not None:
                            add_dep_helper(rd.ins, last_red[k % 2].ins,
                                           reason="rt WAR")
                        rds.append(rd)
                    last_rds[k % 2] = rds

                    mk = mpool.tile([128, Wk * 2], BF16, tag="mk")
                    nc.scalar.dma_start(
                        mk[:], msk[:, 2 * offW : 2 * (offW + Wk)]
                    )
                    nc.vector.tensor_tensor(
                        out=rt[:], in0=rt[:], in1=mk[:],
                        op=mybir.AluOpType.mult,
                    )
                    red = nc.vector.tensor_reduce(
                        out=out_sb[:, k : k + 1],
                        in_=rt[:].rearrange("p (o w) -> p o w", o=1),
                        axis=mybir.AxisListType.X,
                        op=mybir.AluOpType.add,
                    )
                    last_red[k % 2] = red
                    offW += Wk

                nc.sync.dma_start(outd[:, :], out_sb[:])
    nc.compile()
    return nc


_PROGRAM_CACHE = {}


def _get_program(W):
    key = tuple(W)
    if key not in _PROGRAM_CACHE:
        _PROGRAM_CACHE[key] = _build_program(key)
    return _PROGRAM_CACHE[key]


def _preprocess(x, edge_row, edge_col, edge_val):
    """Build per-core idx/mask streams and the row permutation.

    Returns (xT, W, idx_cores, msk_cores, orders) where orders[k] is the
    degree-descending row order of core k (device column-major rank ->
    local row id).
    """
    xT = np.zeros((128, NPAD), ml_dtypes.bfloat16)
    xT[:, :N] = x.T.astype(ml_dtypes.bfloat16)

    order_e = np.argsort(edge_row, kind="stable")
    ers = edge_row[order_e]
    ecs = edge_col[order_e].astype(np.int64)
    evs = edge_val[order_e]

    deg = np.bincount(ers, minlength=N)
    starts = np.zeros(N + 1, np.int64)
    np.cumsum(deg, out=starts[1:])
    pos = (np.arange(E, dtype=np.int64) - starts[ers]).astype(np.int32)

    # per-core degree-sorted rank of each row
    Wk = np.zeros((NCORES, NCHUNK), np.int64)
    ranks = []   # per core: local row id -> rank
    orders = []  # per core: rank -> local row id
    for k in range(NCORES):
        dk = np.zeros(RPAD, np.int64)
        dk[:RPC] = deg[k * RPC : (k + 1) * RPC]
        o = np.argsort(-dk, kind="stable")
        r = np.empty(RPAD, np.int64)
        r[o] = np.arange(RPAD)
        orders.append(o)
        ranks.append(r)
        Wk[k] = dk[o].reshape(NCHUNK, 128).max(axis=1)
    W = Wk.max(axis=0)
    W = np.maximum(W, 1)
    # ap_gather's ucode reads the wrapped int16 idx stream in 4-byte units:
    # keep every chunk's idx column offset even
    W = W + (W & 1)
    sumW = int(W.sum())
    offW = np.zeros(NCHUNK, np.int64)
    np.cumsum(W[:-1], out=offW[1:])

    # per-edge placement
    core = ers // RPC
    rloc = ers - core * RPC
    idx_cores = []
    msk_cores = []
    for k in range(NCORES):
        m = core == k
        rk = ranks[k][rloc[m]]           # rank of dest row
        ck = rk // 128                   # chunk
        P = rk % 128                     # lane/partition
        w = pos[m]                       # slot within row
        cols = ecs[m]
        vals = evs[m]
        ii = (P % 16) * W[ck] + w        # per-gpsimd-core element index
        idx_h = np.zeros((128, sumW), np.int16)
        idx_h[16 * (P // 16) + ii % 16, offW[ck] + ii // 16] = (
            cols >> 1).astype(np.int16)
        msk_h = np.zeros((128, 2 * sumW), ml_dtypes.bfloat16)
        msk_h[P, 2 * offW[ck] + 2 * w + (cols & 1)] = vals.astype(
            ml_dtypes.bfloat16)
        idx_cores.append(idx_h)
        msk_cores.append(msk_h)
    return xT, tuple(int(v) for v in W), idx_cores, msk_cores, orders


def kernel(x, edge_row, edge_col, edge_val, W1, b1, W2, b2):
    x = np.asarray(x, np.float32)
    edge_row = np.asarray(edge_row, np.int32)
    edge_col = np.asarray(edge_col, np.int32)
    edge_val = np.asarray(edge_val, np.float32)
    W1 = np.asarray(W1, np.float32)
    b1 = np.asarray(b1, np.float32)
    W2 = np.asarray(W2, np.float32)
    b2 = np.asarray(b2, np.float32)

    xT, W, idx_cores, msk_cores, orders = _preprocess(
        x, edge_row, edge_col, edge_val
    )
    nc = _get_program(W)

    W1h = np.ascontiguousarray(W1.astype(ml_dtypes.bfloat16))
    b1c = np.tile(b1, 2).reshape(128, 1)
    W2r = np.zeros((128, 256), ml_dtypes.bfloat16)
    W2r[0:64, 0:128] = W2[:, 0:1].astype(ml_dtypes.bfloat16)
    W2r[64:128, 128:256] = W2[:, 0:1].astype(ml_dtypes.bfloat16)

    in_maps = [
        {
            "xT": xT,
            "W1": W1h,
            "b1c": b1c,
            "W2r": W2r,
            "idx": idx_cores[k],
            "msk": msk_cores[k],
        }
        for k in range(NCORES)
    ]
    trace = bool(int(os.environ.get("GCN_TRACE", "0")))
    if trace:
        _install_ntff_hook()
    res = run_bass_kernel_spmd(
        nc,
        in_maps,
        core_ids=list(range(NCORES)),
        trace=trace,
    )
    _LAST_RESULTS["exec_time_ns"] = res.exec_time_ns
    _LAST_RESULTS["trace"] = res.instructions_and_trace
    _LAST_RESULTS["scope_times"] = res.per_core_scope_times

    out = np.empty((N, 1), np.float32)
    b2f = float(b2.reshape(-1)[0])
    for k in range(NCORES):
        o = np.asarray(res.results[k]["out"], np.float32)  # [128, NCHUNK]
        full = np.empty(RPAD, np.float32)
        full[orders[k]] = o.T.reshape(RPAD)  # rank ck*128+P -> row order[rank]
        out[k * RPC : (k + 1) * RPC, 0] = full[:RPC] + b2f
    return out


# revision 15
# speedup vs baseline: 3.9391x; 1.0065x over previous
"""Trainium2 Bass kernel for ClassicalGCN message passing.

Reference computation:
    h   = tanh(x @ W1 + b1)                       # [N, HID]
    agg = segment_sum(edge_val * h[edge_col], edge_row, N)
    out = agg @ W2 + b2                           # [N, 1]

Key algebraic rewrite: W2 commutes through the linear aggregation:

    s      = tanh(x @ W1 + b1) @ W2               # [N] per-node scalar
    out[i] = b2 + sum_{e: row[e]==i} val[e] * s[col[e]]

Sharding: nodes (output rows) are split across the 8 NeuronCores; edges are
partitioned by destination row. x and the small weights are replicated; each
core computes the full s vector locally (no collectives), keeps it in SBUF
as a bf16 pair-table replicated across all 128 partitions, and gathers
per-edge values with the GPSIMD ap_gather ucode (SBUF-local gather; no
per-edge HBM traffic and no SWDGE descriptor generation).

Per-core device program:
  Phase A: s = tanh(x@W1+b1)@W2 for all nodes via PE matmuls, ACT tanh
           (bias fused), PE W2-contraction; each 1024-node chunk is
           converted to bf16 and broadcast from a DRAM staging buffer into
           all 128 partitions of the SBUF table [128, 25088 pairs].
  Phase B: rows are sorted by degree (host-side permutation, undone on the
           host afterwards) and processed in 49 chunks of 128 rows with a
           per-chunk ELL width W[k] = max degree in the chunk. Per chunk:
           ap_gather fetches the bf16 pair containing s[col] for each edge
           slot (idx = col>>1, 8 GPSIMD cores each serve 16 rows), a small
           SBUF->SBUF DMA per 16-partition group re-stripes the (16x
           redundant) gathered stream to one row per partition, and the DVE
           multiplies by a host-built 2-lane mask (edge val at lane col&1)
           and reduces to out[128 rows, chunk].
"""

import os

import numpy as np
import ml_dtypes

import concourse.bass as bass
import concourse.mybir as mybir
import concourse.tile as tile
from concourse import bacc
from concourse.bass_utils import run_bass_kernel_spmd
from concourse.tile_rust import add_dep_helper

# Problem sizes (hardcoded per spec nn_ClassicalGCN_77077483094916)
N = 50000
E = 1600000
IN_DIM = 128
HID = 64
NCORES = 8

RPC = N // NCORES            # rows per core = 6250
RPAD = 6272                  # rows padded to 128*49
NCHUNK = RPAD // 128         # 49 row chunks of 128
NPAD = 50176                 # nodes padded to 49*1024
NPAIR = NPAD // 2            # bf16 pairs in the s table
ACHUNKS = NPAD // 1024       # 49 phase-A iterations

F32 = mybir.dt.float32
BF16 = mybir.dt.bfloat16
I16 = mybir.dt.int16

_LAST_RESULTS = {"exec_time_ns": None}


def _install_ntff_hook():
    """Register the axon NTFF profile hook if the image's antenv lacks it.

    bass_utils reads `antenv.axon_hooks.get_axon_ntff_profile_hook` lazily
    when trace=True under axon; this image's antenv has no axon_hooks
    module, so synthesize one around direct ctypes calls into
    libaxon_pjrt.so (same ABI trn_boot._ntff_profile_via_ctypes uses).
    """
    import sys

    if "antenv.axon_hooks" in sys.modules:
        return
    import contextlib
    import ctypes
    import types

    so_path = "/opt/axon/libaxon_pjrt.so"
    if not os.path.exists(so_path):
        return
    try:
        lib = ctypes.CDLL(so_path)
    except OSError:
        return
    if not hasattr(lib, "axon_start_nrt_profile"):
        return
    lib.axon_start_nrt_profile.argtypes = [
        ctypes.POINTER(ctypes.c_int64),
        ctypes.c_size_t,
    ]
    lib.axon_start_nrt_profile.restype = ctypes.c_int64
    lib.axon_stop_nrt_profile.argtypes = [ctypes.c_char_p]
    lib.axon_stop_nrt_profile.restype = ctypes.c_int64

    @contextlib.contextmanager
    def _hook(output_dir, device_ids):
        import jax

        jax.devices()
        if device_ids:
            ids = (ctypes.c_int64 * len(device_ids))(*device_ids)
            rc = lib.axon_start_nrt_profile(ids, len(device_ids))
        else:
            rc = lib.axon_start_nrt_profile(None, 0)
        if rc != 0:
            raise RuntimeError(f"axon_start_nrt_profile rc={rc}")
        try:
            yield
        finally:
            n = lib.axon_stop_nrt_profile(str(output_dir).encode())
            if n < 0:
                raise RuntimeError(f"axon_stop_nrt_profile rc={n}")
            print(f"profile: {n} file(s) written to {output_dir}")

    mod = types.ModuleType("antenv.axon_hooks")
    mod.get_axon_ntff_profile_hook = lambda: _hook
    mod.set_axon_ntff_profile_hook = lambda h: None
    sys.modules["antenv.axon_hooks"] = mod


def _build_program(W):
    """W: tuple of NCHUNK per-chunk ELL widths (max degree in each chunk)."""
    sumW = sum(W)
    nc = bacc.Bacc("TRN2", target_bir_lowering=False, debug=False)

    xT = nc.dram_tensor("xT", [128, NPAD], BF16, kind="ExternalInput")
    W1 = nc.dram_tensor("W1", [128, HID], BF16, kind="ExternalInput")
    b1c = nc.dram_tensor("b1c", [128, 1], F32, kind="ExternalInput")
    # W2r[:, 0:128]: col m = W2 on partitions 0:64, zeros elsewhere;
    # W2r[:, 128:256]: col m = W2 on partitions 64:128. Contracting either
    # against th replicates the per-node scalar across all 128 partitions.
    W2r = nc.dram_tensor("W2r", [128, 256], BF16, kind="ExternalInput")
    idx = nc.dram_tensor("idx", [128, sumW], I16, kind="ExternalInput")
    msk = nc.dram_tensor("msk", [128, 2 * sumW], BF16, kind="ExternalInput")
    outd = nc.dram_tensor("out", [128, NCHUNK], F32, kind="ExternalOutput")

    with tile.TileContext(nc) as tc:
        with (
            tc.tile_pool(name="const", bufs=1) as cpool,
        ):
            W1_sb = cpool.tile([128, HID], BF16)
            nc.sync.dma_start(W1_sb[:], W1[:, :])
            b1_sb = cpool.tile([128, 1], F32)
            nc.sync.dma_start(b1_sb[:], b1c[:, :])
            W2_sb = cpool.tile([128, 256], BF16)
            nc.sync.dma_start(W2_sb[:], W2r[:, :])

            table_sb = cpool.tile([128, NPAD], BF16)

            # Dummy ap_gather issued first: pays the ~6us GPSIMD library
            # IRAM load during phase A instead of on the critical path.
            dtbl = cpool.tile([128, 32], BF16)
            m0 = nc.vector.memset(dtbl[:], 0)
            didx = cpool.tile([128, 1], I16)
            m1 = nc.vector.memset(didx[:], 0)
            dout = cpool.tile([128, 32], BF16)
            dg = nc.gpsimd.ap_gather(
                out_ap=dout[:].rearrange("p (n d) -> p n d", d=2),
                in_ap=dtbl[:].rearrange("p (n d) -> p n d", d=2),
                idxs_ap=didx[:, :],
                channels=128,
                num_elems=16,
                d=2,
                num_idxs=16,
            )
            add_dep_helper(dg.ins, m0.ins, reason="dummy tbl init")
            add_dep_helper(dg.ins, m1.ins, reason="dummy idx init")

            # ---- Phase A: table[:, n] = tanh(x@W1+b1)@W2, all partitions --
            bcast_insts = []
            with (
                tc.tile_pool(name="xload", bufs=3) as xpool,
                tc.tile_pool(name="thp", bufs=2) as thpool,
                tc.tile_pool(name="pz", bufs=2, space="PSUM") as pz,
                tc.tile_pool(name="psd", bufs=2, space="PSUM") as psd,
            ):
                for i in range(ACHUNKS):
                    xt = xpool.tile([128, 1024], BF16)
                    nc.sync.dma_start(xt[:], xT[:, 1024 * i : 1024 * (i + 1)])
                    z = pz.tile([128, 512], F32)
                    nc.tensor.matmul(z[0:64, :], lhsT=W1_sb[:],
                                     rhs=xt[:, 0:512], start=True, stop=True)
                    nc.tensor.matmul(z[64:128, :], lhsT=W1_sb[:],
                                     rhs=xt[:, 512:1024], start=True, stop=True)
                    th = thpool.tile([128, 512], BF16)
                    nc.scalar.activation(th[:], z[:],
                                         mybir.ActivationFunctionType.Tanh,
                                         bias=b1_sb[:, 0:1])
                    sA = psd.tile([128, 512], F32, tag="sA")
                    nc.tensor.matmul(sA[:], lhsT=W2_sb[:, 0:128], rhs=th[:],
                                     start=True, stop=True)
                    sB = psd.tile([128, 512], F32, tag="sB")
                    nc.tensor.matmul(sB[:], lhsT=W2_sb[:, 128:256], rhs=th[:],
                                     start=True, stop=True)
                    cA = nc.scalar.activation(
                        table_sb[:, 1024 * i : 1024 * i + 512], sA[:],
                        mybir.ActivationFunctionType.Copy)
                    cB = nc.vector.tensor_copy(
                        table_sb[:, 1024 * i + 512 : 1024 * (i + 1)], sB[:])
                    bcast_insts.append(cA)
                    bcast_insts.append(cB)

            tblv = table_sb[:].rearrange("p (n d) -> p n d", d=2)

            # ---- Phase B: per-chunk ap_gather + restripe + mask-reduce ----
            with (
                tc.tile_pool(name="gat", bufs=3) as gpool,
                tc.tile_pool(name="rts", bufs=2) as rpool,
                tc.tile_pool(name="mks", bufs=2) as mpool,
                tc.tile_pool(name="ell", bufs=1) as epool,
            ):
                idx_sb = epool.tile([128, sumW], I16)
                idx_ld = nc.sync.dma_start(idx_sb[:], idx[:, :])
                out_sb = epool.tile([128, NCHUNK], F32)

                offW = 0
                last_rds = [None, None, None]  # per g slot: restripe DMAs
                last_red = [None, None]        # per rt slot: reduce inst
                for k in range(NCHUNK):
                    Wk = W[k]
                    ni = 16 * Wk
                    g = gpool.tile([128, ni * 2], BF16, tag="g")
                    gi = nc.gpsimd.ap_gather(
                        out_ap=g[:].rearrange("p (n d) -> p n d", d=2),
                        in_ap=tblv,
                        idxs_ap=idx_sb[:, offW : offW + Wk],
                        channels=128,
                        num_elems=NPAIR,
                        d=2,
                        num_idxs=ni,
                    )
                    if k == 0:
                        for bi in bcast_insts:
                            add_dep_helper(gi.ins, bi.ins, reason="table RAW")
                        add_dep_helper(gi.ins, idx_ld.ins, reason="idx RAW")
                    if last_rds[k % 3] is not None:
                        for rd in last_rds[k % 3]:
                            add_dep_helper(gi.ins, rd.ins, reason="g WAR")

                    rt = rpool.tile([128, Wk * 2], BF16, tag="rt")
                    rds = []
                    gv = g[:].rearrange("p (s wl) -> p s wl", s=16)
                    for c in range(8):
                        rd = nc.sync.dma_start(
                            rt[16 * c : 16 * (c + 1), :],
                            gv[16 * c : 16 * c + 1, :, :],
                        )
                        add_dep_helper(rd.ins, gi.ins, reason="gather RAW")
                        if last_red[k % 2] is not None:
                            add_dep_helper(rd.ins, last_red[k % 2].ins,
                                           reason="rt WAR")
                        rds.append(rd)
                    last_rds[k % 3] = rds

                    mk = mpool.tile([128, Wk * 2], BF16, tag="mk")
                    nc.scalar.dma_start(
                        mk[:], msk[:, 2 * offW : 2 * (offW + Wk)]
                    )
                    nc.vector.tensor_tensor(
                        out=rt[:], in0=rt[:], in1=mk[:],
                        op=mybir.AluOpType.mult,
                    )
                    red = nc.vector.tensor_reduce(
                        out=out_sb[:, k : k + 1],
                        in_=rt[:].rearrange("p (o w) -> p o w", o=1),
                        axis=mybir.AxisListType.X,
                        op=mybir.AluOpType.add,
                    )
                    last_red[k % 2] = red
                    offW += Wk

                nc.sync.dma_start(outd[:, :], out_sb[:])
    nc.compile()
    return nc


_PROGRAM_CACHE = {}


def _get_program(W):
    key = tuple(W)
    if key not in _PROGRAM_CACHE:
        _PROGRAM_CACHE[key] = _build_program(key)
    return _PROGRAM_CACHE[key]


def _preprocess(x, edge_row, edge_col, edge_val):
    """Build per-core idx/mask streams and the row permutation.

    Returns (xT, W, idx_cores, msk_cores, orders) where orders[k] is the
    degree-descending row order of core k (device column-major rank ->
    local row id).
    """
    xT = np.zeros((128, NPAD), ml_dtypes.bfloat16)
    xT[:, :N] = x.T.astype(ml_dtypes.bfloat16)

    order_e = np.argsort(edge_row, kind="stable")
    ers = edge_row[order_e]
    ecs = edge_col[order_e].astype(np.int64)
    evs = edge_val[order_e]

    deg = np.bincount(ers, minlength=N)
    starts = np.zeros(N + 1, np.int64)
    np.cumsum(deg, out=starts[1:])
    pos = (np.arange(E, dtype=np.int64) - starts[ers]).astype(np.int32)

    # per-core degree-sorted rank of each row
    Wk = np.zeros((NCORES, NCHUNK), np.int64)
    ranks = []   # per core: local row id -> rank
    orders = []  # per core: rank -> local row id
    for k in range(NCORES):
        dk = np.zeros(RPAD, np.int64)
        dk[:RPC] = deg[k * RPC : (k + 1) * RPC]
        o = np.argsort(-dk, kind="stable")
        r = np.empty(RPAD, np.int64)
        r[o] = np.arange(RPAD)
        orders.append(o)
        ranks.append(r)
        Wk[k] = dk[o].reshape(NCHUNK, 128).max(axis=1)
    W = Wk.max(axis=0)
    W = np.maximum(W, 1)
    # ap_gather's ucode reads the wrapped int16 idx stream in 4-byte units:
    # keep every chunk's idx column offset even
    W = W + (W & 1)
    sumW = int(W.sum())
    offW = np.zeros(NCHUNK, np.int64)
    np.cumsum(W[:-1], out=offW[1:])

    # per-edge placement
    core = ers // RPC
    rloc = ers - core * RPC
    idx_cores = []
    msk_cores = []
    for k in range(NCORES):
        m = core == k
        rk = ranks[k][rloc[m]]           # rank of dest row
        ck = rk // 128                   # chunk
        P = rk % 128                     # lane/partition
        w = pos[m]                       # slot within row
        cols = ecs[m]
        vals = evs[m]
        ii = (P % 16) * W[ck] + w        # per-gpsimd-core element index
        idx_h = np.zeros((128, sumW), np.int16)
        idx_h[16 * (P // 16) + ii % 16, offW[ck] + ii // 16] = (
            cols >> 1).astype(np.int16)
        msk_h = np.zeros((128, 2 * sumW), ml_dtypes.bfloat16)
        msk_h[P, 2 * offW[ck] + 2 * w + (cols & 1)] = vals.astype(
            ml_dtypes.bfloat16)
        idx_cores.append(idx_h)
        msk_cores.append(msk_h)
    return xT, tuple(int(v) for v in W), idx_cores, msk_cores, orders


def kernel(x, edge_row, edge_col, edge_val, W1, b1, W2, b2):
    x = np.asarray(x, np.float32)
    edge_row = np.asarray(edge_row, np.int32)
    edge_col = np.asarray(edge_col, np.int32)
    edge_val = np.asarray(edge_val, np.float32)
    W1 = np.asarray(W1, np.float32)
    b1 = np.asarray(b1, np.float32)
    W2 = np.asarray(W2, np.float32)
    b2 = np.asarray(b2, np.float32)

    xT, W, idx_cores, msk_cores, orders = _preprocess(
        x, edge_row, edge_col, edge_val
    )
    nc = _get_program(W)

    W1h = np.ascontiguousarray(W1.astype(ml_dtypes.bfloat16))
    b1c = np.tile(b1, 2).reshape(128, 1)
    W2r = np.zeros((128, 256), ml_dtypes.bfloat16)
    W2r[0:64, 0:128] = W2[:, 0:1].astype(ml_dtypes.bfloat16)
    W2r[64:128, 128:256] = W2[:, 0:1].astype(ml_dtypes.bfloat16)

    in_maps = [
        {
            "xT": xT,
            "W1": W1h,
            "b1c": b1c,
            "W2r": W2r,
            "idx": idx_cores[k],
            "msk": msk_cores[k],
        }
        for k in range(NCORES)
    ]
    trace = bool(int(os.environ.get("GCN_TRACE", "0")))
    if trace:
        _install_ntff_hook()
    res = run_bass_kernel_spmd(
        nc,
        in_maps,
        core_ids=list(range(NCORES)),
        trace=trace,
    )
    _LAST_RESULTS["exec_time_ns"] = res.exec_time_ns
    _LAST_RESULTS["trace"] = res.instructions_and_trace
    _LAST_RESULTS["scope_times"] = res.per_core_scope_times

    out = np.empty((N, 1), np.float32)
    b2f = float(b2.reshape(-1)[0])
    for k in range(NCORES):
        o = np.asarray(res.results[k]["out"], np.float32)  # [128, NCHUNK]
        full = np.empty(RPAD, np.float32)
        full[orders[k]] = o.T.reshape(RPAD)  # rank ck*128+P -> row order[rank]
        out[k * RPC : (k + 1) * RPC, 0] = full[:RPC] + b2f
    return out
